# revision 17
# baseline (speedup 1.0000x reference)
"""Trainium2 Bass kernel for ModLinear forward:

    alpha = z @ weight_alpha.T + bias_alpha          # [B, IN]
    beta  = z @ weight_beta.T  + bias_beta           # [B, OUT]
    out   = (x * alpha[:, None, :]) @ weight.T + beta[:, None, :]

Restructuring:
  * alpha folds into the weight per batch: out[b] = x[b] @ (W.T * alpha[b][:,None]) + beta[b].
  * The 2e-2 rel-err budget admits fp16 for x / W / out (measured err ~4e-4),
    halving HBM traffic vs f32: 64 MiB per core instead of 128 MiB.
  * Host pre-transposes x to feature-major tiles, so the device does *no* PE
    transposes and no PSUM->SBUF staging copies: pure matmul.
  * Output is produced transposed ([out_feat, rows] per core); host untransposes.

Sharding: x flattened to [B*N, IN] = [262144, 512], split into 8 contiguous
row blocks (batch boundary falls between cores 3 and 4, so each core uses a
single (wmodT, beta) pair). No cross-core communication.

Device kernel per core (rows = 32768), half-superblock hh = 512 rows:
  DMA xT half [128, 4ic x 512n] fp16 (4 KiB/partition contiguous) -> SBUF
  4 out-chunks x 4 in-chunks:
    PE matmul po[oc] += wmod[ic,oc].T @ xT[ic]   (fp16 operands, f32 PSUM)
  epilogue, split DVE (oc 0,1) / ACT (oc 2,3): po + beta[oc] -> fp16 SBUF
  DMA outT half [128, 4oc x 512n] fp16 -> DRAM (second HWDGE ring)

Head: W and the first x half are 2-way split so the stream starts on the
first 640 KiB instead of the full 1.06 MiB; block 0 runs ic-major with 4
live PSUM banks so its first matmuls need only the first chunks. Warm-up
matmuls (gpsimd-memset dummy, 256-col) start right after the preamble and
ramp the PE HAM clock out of the low pstate before real work arrives.
Tail: the last half is drained as two 256-row sub-blocks, each stored with
a single strided DMA, so the post-stream epilogue covers 256 rows not 512.

fp8 is a dead end under the max-abs gate: e4m3 DoubleRow measures 3.5e-2
(> 2e-2), e3m4 runs at 1.0 cycles/row (no DoubleRow per the hw decode),
and no hi/lo slot scheme beats fp16's 4 passes (slot accounting: gate
needs err^2 <= 0.33 sigma^2 which costs >= 9.4 fp8 slots ~ 5 passes).
The fp16 PE-stream floor is 524288 columns through a 128x128 array.
"""

import numpy as np

B, N = 2, 131072
IN_F, OUT_F, STYLE_F = 512, 512, 256
NCORES = 8
ROWS = B * N
ROWS_PER_CORE = ROWS // NCORES  # 32768
P = 128
HB = 512                        # rows per half-superblock (= matmul free dim)
NHALF = ROWS_PER_CORE // HB     # 64
NIC = IN_F // P                 # 4 input-feature chunks
NOC = OUT_F // P                # 4 output-feature chunks
NWARM = 12                      # HAM warm-up matmuls (256-col)
TAILS = 256                     # tail sub-block rows
NSUB = HB // TAILS              # sub-blocks per half (out-layout granularity)
NFP8 = 2                        # head blocks with x in fp8 e3m4 (halves head bytes)


def _build_body(tc, out_ap, x8_ap, x_ap, w_ap, beta_ap):
    from concourse import mybir

    nc = tc.nc
    f32 = mybir.dt.float32
    f16 = mybir.dt.float16
    f8 = mybir.dt.float8e3

    # dram x (blocks NFP8..): [P, S, NIC, HB] -> per half [P, NIC*HB]
    # (per partition per half: one contiguous 4 KiB run)
    x_v = x_ap.rearrange("p s c n -> s p (c n)")
    # dram x8 (blocks 0..NFP8-1, e3m4): halves the head bytes — the first
    # ~8us of DMA delivery runs at ~230 GB/s (the engines have their own
    # activity ramp), so the head blocks are delivery-paced and bytes are
    # the binding cost. x-side e3m4 on these 2*HB rows measures 1.2e-2
    # (host sim) vs the 2e-2 gate; the other 99.2% of rows stay at 4e-4.
    x8_v = x8_ap.rearrange("p s c n -> s p (c n)")
    # dram out: [P, NHALF, NSUB, NOC, TAILS] -> per half [P, NSUB*NOC*TAILS].
    # Sub-block-major so the tail sub-stores are contiguous on both sides.
    out_v = out_ap.rearrange("p s u c n -> s p (u c n)")

    with (
        tc.tile_pool(name="const", bufs=1) as cpool,
        tc.tile_pool(name="xin", bufs=8) as xpool,
        tc.tile_pool(name="oout", bufs=8) as opool,
        tc.tile_pool(name="pmm", bufs=8, space="PSUM") as pmpool,
    ):
        # Warm-up matmuls on a gpsimd-memset dummy: gpsimd is free right
        # after the preamble (~6.5us) while DVE is still loading tables, so
        # the PE starts ramping ~1.5us earlier than with a DVE memset.
        # 256-col warm-ups keep the queue short enough that the last one
        # drains before the first x chunk lands.
        dummy = cpool.tile([P, 256], f16)
        nc.gpsimd.memset(dummy[:], 0.0)
        dpo = pmpool.tile([P, HB], f32, name="po", tag="po")
        for _ in range(NWARM):
            nc.tensor.matmul(dpo[:, :256], dummy[:, :P], dummy[:], start=True, stop=True)

        # W on the ACT HWDGE ring (the sync ring starts on x), split so
        # delivery lands in consumption order: the very first matmul needs
        # only W(ic0,oc0) (32 KiB), so that ships alone, then the rest of
        # ic0, then per-ic chunks. The head DMA rate is ~230 GB/s (the
        # engines have their own activity ramp), so gating bytes directly
        # set the stream start.
        w_sb = cpool.tile([P, NIC * OUT_F], f16)
        nc.scalar.dma_start(out=w_sb[:, :P], in_=w_ap[:, :P])
        nc.scalar.dma_start(out=w_sb[:, P:OUT_F], in_=w_ap[:, P:OUT_F])
        for ic in range(1, NIC):
            nc.scalar.dma_start(
                out=w_sb[:, ic * OUT_F : (ic + 1) * OUT_F],
                in_=w_ap[:, ic * OUT_F : (ic + 1) * OUT_F],
            )
        beta_sb = cpool.tile([P, NOC], f32)
        nc.scalar.dma_start(out=beta_sb[:], in_=beta_ap[:, :])

        def epilogue(po_ap, osl, oc):
            if oc < 2:
                # Epilogue split DVE/ACT halves the PSUM drain latency
                nc.vector.tensor_scalar_add(
                    out=osl, in0=po_ap, scalar1=beta_sb[:, oc : oc + 1],
                )
            else:
                nc.scalar.add(osl, po_ap, beta_sb[:, oc : oc + 1])

        def ot_view(ot):
            # SBUF view matching the dram [u, c, n] sub-block-major layout
            return ot[:].rearrange("p (u c n) -> p u c n", u=NSUB, c=NOC)

        # ---- head blocks 0..NFP8: delivery-paced, so (a) x in e3m4 for the
        # first NFP8 blocks halves their bytes, (b) per-chunk DMAs +
        # ic-major matmuls (4 live PSUM banks) let consumption track the
        # delivery order — matmul ic_k needs only x chunk k + W chunk k —
        # instead of waiting for a whole-half completion semaphore.
        for s in range(NFP8 + 1):
            fp8 = s < NFP8
            src = x8_v[s] if fp8 else x_v[s - NFP8]
            xt = xpool.tile([P, NIC * HB], f8 if fp8 else f16)
            if s == 0:
                # block 0's first chunk ships as two 64 KiB pieces so the
                # first matmul is gated on 96 KiB (x c0a + W ic0oc0), not 256
                splits = [HB // 2, HB, 2 * HB, 3 * HB, 4 * HB]
            else:
                splits = [2 * HB, 4 * HB] if not fp8 else [HB, 2 * HB, 3 * HB, 4 * HB]
            lo = 0
            for hi in splits:
                nc.sync.dma_start(out=xt[:, lo:hi], in_=src[:, lo:hi])
                lo = hi
            pos = [pmpool.tile([P, HB], f32, name="po", tag="po") for _ in range(NOC)]
            for ic in range(NIC):
                for oc in range(NOC):
                    nc.tensor.matmul(
                        pos[oc][:],
                        w_sb[:, ic * OUT_F + oc * P : ic * OUT_F + (oc + 1) * P],
                        xt[:, ic * HB : (ic + 1) * HB],
                        start=(ic == 0),
                        stop=(ic == NIC - 1),
                    )
            ot = opool.tile([P, NOC * HB], f16)
            ov = ot_view(ot)
            for oc in range(NOC):
                epilogue(
                    pos[oc][:].rearrange("p (u n) -> p u n", u=NSUB), ov[:, :, oc, :], oc
                )
            nc.scalar.dma_start(out=out_v[s], in_=ot[:])

        # ---- steady state: blocks NFP8+1..NHALF-2, per-oc accumulation groups
        for s in range(NFP8 + 1, NHALF - 1):
            xt = xpool.tile([P, NIC * HB], f16)
            nc.sync.dma_start(out=xt[:], in_=x_v[s - NFP8])
            ot = opool.tile([P, NOC * HB], f16)
            ov = ot_view(ot)
            for oc in range(NOC):
                po = pmpool.tile([P, HB], f32, name="po", tag="po")
                for ic in range(NIC):
                    nc.tensor.matmul(
                        po[:],
                        w_sb[:, ic * OUT_F + oc * P : ic * OUT_F + (oc + 1) * P],
                        xt[:, ic * HB : (ic + 1) * HB],
                        start=(ic == 0),
                        stop=(ic == NIC - 1),
                    )
                epilogue(po[:].rearrange("p (u n) -> p u n", u=NSUB), ov[:, :, oc, :], oc)
            nc.scalar.dma_start(out=out_v[s], in_=ot[:])

        # ---- last block: NSUB 256-row sub-blocks so the post-stream drain
        # (epilogue + store + fence) covers a quarter of the bytes. The
        # sub-block-major out layout makes each sub-store a single DMA that
        # is contiguous on both SBUF and DRAM sides (2 KiB runs).
        s = NHALF - 1
        xt = xpool.tile([P, NIC * HB], f16)
        nc.sync.dma_start(out=xt[:], in_=x_v[s - NFP8])
        ot = opool.tile([P, NOC * HB], f16)
        ov = ot_view(ot)
        sub_bytes = NOC * TAILS
        for sub in range(NSUB):
            n0 = sub * TAILS
            last = sub == NSUB - 1
            # Final sub: close groups in order [2,3,0,1] so the last-closed
            # group drains on DVE (its ACT peers finished mid-stream), and
            # the oc23 half-store issues while the oc01 groups still stream.
            oc_order = (2, 3, 0, 1) if last else range(NOC)
            for j, oc in enumerate(oc_order):
                po = pmpool.tile([P, HB], f32, name="po", tag="po")
                for ic in range(NIC):
                    nc.tensor.matmul(
                        po[:, :TAILS],
                        w_sb[:, ic * OUT_F + oc * P : ic * OUT_F + (oc + 1) * P],
                        xt[:, ic * HB + n0 : ic * HB + n0 + TAILS],
                        start=(ic == 0),
                        stop=(ic == NIC - 1),
                    )
                epilogue(po[:, :TAILS], ov[:, sub, oc, :], oc)
                if last and j == 1:
                    # oc2,oc3 epilogues done -> store the back half early
                    nc.scalar.dma_start(
                        out=out_v[s][:, sub * sub_bytes + 2 * TAILS : (sub + 1) * sub_bytes],
                        in_=ot[:, sub * sub_bytes + 2 * TAILS : (sub + 1) * sub_bytes],
                    )
            if last:
                nc.scalar.dma_start(
                    out=out_v[s][:, sub * sub_bytes : sub * sub_bytes + 2 * TAILS],
                    in_=ot[:, sub * sub_bytes : sub * sub_bytes + 2 * TAILS],
                )
            else:
                nc.scalar.dma_start(
                    out=out_v[s][:, sub * sub_bytes : (sub + 1) * sub_bytes],
                    in_=ot[:, sub * sub_bytes : (sub + 1) * sub_bytes],
                )


def build_nc():
    """Build + compile the per-core Bass program."""
    import concourse.tile as tile
    from concourse import bacc, mybir

    f32 = mybir.dt.float32
    f16 = mybir.dt.float16
    f8 = mybir.dt.float8e3
    nc = bacc.Bacc(
        "TRN2", target_bir_lowering=False, debug=False, num_devices=NCORES
    )
    x8_t = nc.dram_tensor("x8", [P, NFP8, NIC, HB], f8, kind="ExternalInput")
    x_t = nc.dram_tensor("x", [P, NHALF - NFP8, NIC, HB], f16, kind="ExternalInput")
    w_t = nc.dram_tensor("wt", [P, NIC * OUT_F], f16, kind="ExternalInput")
    beta_t = nc.dram_tensor("beta", [P, NOC], f32, kind="ExternalInput")
    out_t = nc.dram_tensor(
        "out", [P, NHALF, NSUB, NOC, TAILS], f16, kind="ExternalOutput"
    )

    with tile.TileContext(nc) as tc:
        _build_body(tc, out_t.ap(), x8_t.ap(), x_t.ap(), w_t.ap(), beta_t.ap())
    nc.compile()
    return nc


_NC_CACHE = {}


def _get_nc():
    if "nc" not in _NC_CACHE:
        _NC_CACHE["nc"] = build_nc()
    return _NC_CACHE["nc"]


def host_prep(x, z, weight, weight_alpha, bias_alpha, weight_beta, bias_beta):
    """Fold alpha into W, quantize to fp16, pre-transpose/tile x per core."""
    z64 = z.astype(np.float64)
    alpha = (z64 @ weight_alpha.astype(np.float64).T) + bias_alpha.astype(np.float64)
    beta = (z64 @ weight_beta.astype(np.float64).T) + bias_beta.astype(np.float64)
    alpha = alpha.astype(np.float32)  # [B, IN_F]
    beta = beta.astype(np.float32)  # [B, OUT_F]

    # w_sb[p, ic*OUT_F + o] = weight[o, ic*P + p] * alpha[ic*P + p]
    wmod = [
        np.ascontiguousarray(
            (weight.T * alpha[b][:, None])
            .reshape(NIC, P, OUT_F)
            .transpose(1, 0, 2)
            .reshape(P, NIC * OUT_F)
        ).astype(np.float16)
        for b in range(B)
    ]
    # beta rearranged [P, NOC]: beta_r[p, oc] = beta[oc*P + p]
    beta_r = [
        np.ascontiguousarray(beta[b].reshape(NOC, P).T).astype(np.float32)
        for b in range(B)
    ]

    # x: [ROWS, IN_F] f32 -> per core [P, S, NIC, HB] with
    # xk[p, s, c, n] = x[core_base + s*HB + n, c*P + p]; the first NFP8
    # halves ship as e3m4 (x8), the rest as fp16 (x).
    import ml_dtypes

    xp = x.reshape(NCORES, NHALF, HB, NIC, P)
    in_maps = []
    for k in range(NCORES):
        b = (k * ROWS_PER_CORE) // N  # batch this core's rows belong to
        in_maps.append(
            {
                "x8": np.ascontiguousarray(
                    xp[k, :NFP8].transpose(3, 0, 2, 1).astype(ml_dtypes.float8_e3m4)
                ),
                "x": np.ascontiguousarray(
                    xp[k, NFP8:].transpose(3, 0, 2, 1).astype(np.float16)
                ),
                "wt": wmod[b],
                "beta": beta_r[b],
            }
        )
    return in_maps


def kernel(x, z, weight, weight_alpha, bias_alpha, weight_beta, bias_beta,
           _trace=False):
    from concourse.bass_utils import run_bass_kernel_spmd

    x = np.asarray(x, dtype=np.float32).reshape(ROWS, IN_F)
    z = np.asarray(z, dtype=np.float32)
    weight = np.asarray(weight, dtype=np.float32)
    weight_alpha = np.asarray(weight_alpha, dtype=np.float32)
    bias_alpha = np.asarray(bias_alpha, dtype=np.float32)
    weight_beta = np.asarray(weight_beta, dtype=np.float32)
    bias_beta = np.asarray(bias_beta, dtype=np.float32)
    in_maps = host_prep(
        x, z, weight, weight_alpha, bias_alpha, weight_beta, bias_beta
    )
    nc = _get_nc()
    res = run_bass_kernel_spmd(
        nc, in_maps, core_ids=list(range(NCORES)), trace=_trace
    )
    # out dram [P, NHALF, NSUB, NOC, TAILS] fp16: row = s*HB + u*TAILS + n,
    # feature = c*P + p -> rows [ROWS_PER_CORE, OUT_F] f32
    out = np.empty((ROWS, OUT_F), dtype=np.float32)
    for k in range(NCORES):
        o = res.results[k]["out"]  # [P, NHALF, NSUB, NOC, TAILS] fp16
        out[k * ROWS_PER_CORE : (k + 1) * ROWS_PER_CORE] = (
            np.asarray(o).transpose(1, 2, 4, 3, 0).reshape(ROWS_PER_CORE, OUT_F)
        )
    out = out.reshape(B, N, OUT_F)
    if _trace:
        kernel.last_results = res
    return out
